# revision 16
# baseline (speedup 1.0000x reference)
"""Trainium2 Bass kernel for nn_DomainAdapter (moe_routing).

Reference computation (per sample b with expert e = domain_id[b]):
    h  = gelu(x @ down_W.T + down_b)                  # [S, A]
    h  = where(valid, h @ W_e.T + b_e, h) + emb[e]    # per-sample expert
    y  = LayerNorm(h @ up_W.T + up_b + x) * gamma + beta

Strategy (8 NeuronCores, data-parallel over batch; 4 samples/core):
  - host: gather per-sample expert weights (invalid ids -> identity W, so
    the on-device math is uniform), fold up_b into the residual stream
    (x'' = x + up_b with down_b' = down_b - down_W @ up_b so the down
    projection is unchanged), transpose x'' to xT [D, T] per core, cast
    everything on the matmul path to bf16
  - device: all matmuls in bf16 (1 cycle/row on the PE at any tile size;
    fp32r pays 4x on free-dim<256 outputs, which the residual-transpose
    identity matmuls would otherwise hit)
      down:   h1.T[a,t] = dwT.T @ xT       (xT tiles straight from DRAM)
      gelu:   ACT Gelu LUT (erf-exact) + down_b' as per-partition bias
      expert: per-sample W.T as stationary tiles (compile-time indexed)
      up:     accumulated in PSUM together with the residual (identity
              matmuls from xT chunks; up_b already folded into xT)
      LN:     bn_stats/bn_aggr on DVE, rsqrt via fast-inverse-sqrt + 2
              Newton steps on DVE, normalize on ACT (scale/bias form),
              bf16 output
  - ln_gamma/ln_beta applied on host only if non-trivial (they are 1/0)
"""
import numpy as np

from concourse import bacc, tile, mybir
from concourse.bass2jax import (
    _bass_exec_p,
    install_neuronx_cc_hook,
    partition_id_tensor,
)

f32 = mybir.dt.float32
bf16 = mybir.dt.bfloat16
i32 = mybir.dt.int32
AF = mybir.ActivationFunctionType
ALU = mybir.AluOpType

B, S, D, A, E = 32, 1024, 1024, 256, 16
N_CORES = 8
NS = B // N_CORES          # samples per core = 4
T = NS * S                 # tokens per core = 4096
GT = 512                   # tokens per group
NG = T // GT               # 8 groups
TS = 4                     # 128-token sub-tiles per group
KD = D // 128              # 8
KA = A // 128              # 2
DH = D // 512              # 2
LN_EPS = 1e-5
FISR_C = 0x5F3759DF


def _build_nc(repeat=1):
    nc = bacc.Bacc("TRN2", target_bir_lowering=False, debug=False)

    XT = nc.dram_tensor("xt", [NG, 128, KD, GT], bf16, kind="ExternalInput").ap()
    DWT = nc.dram_tensor("dwt", [D, A], bf16, kind="ExternalInput").ap()
    DB = nc.dram_tensor("db", [128, KA], f32, kind="ExternalInput").ap()
    WET = nc.dram_tensor("wet", [NS, A, A], bf16, kind="ExternalInput").ap()
    BEME = nc.dram_tensor("beme", [128, NS * KA], f32, kind="ExternalInput").ap()
    UPT = nc.dram_tensor("upt", [A, D], bf16, kind="ExternalInput").ap()
    IDENT = nc.dram_tensor("ident", [128, 128], bf16, kind="ExternalInput").ap()
    Y = nc.dram_tensor("y", [T, D], bf16, kind="ExternalOutput").ap()

    with tile.TileContext(nc) as tc:
        with (
            tc.tile_pool(name="consts", bufs=1) as cpool,
            tc.tile_pool(name="xTp", bufs=6) as xT_pool,
            tc.tile_pool(name="gh", bufs=2) as gh_pool,
            tc.tile_pool(name="outp", bufs=3) as out_pool,
            tc.tile_pool(name="stats", bufs=10) as st_pool,
            tc.tile_pool(name="pmid", bufs=2, space="PSUM") as pmid_pool,
            tc.tile_pool(name="px", bufs=6, space="PSUM") as px_pool,
        ):
            dwt = cpool.tile([128, KD, A], bf16, tag="dwt")
            dwt_src = DWT.rearrange("(k p) a -> p k a", p=128)
            nc.sync.dma_start(dwt[:, 0:2], dwt_src[:, 0:2])
            nc.sync.dma_start(dwt[:, 2:KD], dwt_src[:, 2:KD])
            db = cpool.tile([128, KA], f32, tag="db")
            nc.scalar.dma_start(db[:], DB)
            ident = cpool.tile([128, 128], bf16, tag="ident")
            nc.scalar.dma_start(ident[:], IDENT)
            wet = cpool.tile([128, NS, KA, A], bf16, tag="wet")
            nc.sync.dma_start(wet[:], WET.rearrange("s (k p) a -> p s k a", p=128))
            upt = cpool.tile([128, KA, D], bf16, tag="upt")
            nc.sync.dma_start(upt[:], UPT.rearrange("(k p) d -> p k d", p=128))
            beme = cpool.tile([128, NS * KA], f32, tag="beme")
            nc.scalar.dma_start(beme[:], BEME)

            state = {}

            def front(rep, g):
                s = g // 2
                xTg = xT_pool.tile([128, KD, GT], bf16, tag="xTg",
                                   name=f"xTg_{rep}_{g}")
                if rep == 0 and g == 0:
                    # split the very first load per contraction chunk so the
                    # first down matmuls start ~3.5us earlier
                    for kd in range(KD):
                        nc.gpsimd.dma_start(xTg[:, kd:kd + 1], XT[g, :, kd:kd + 1])
                else:
                    nc.gpsimd.dma_start(xTg[:], XT[g])

                ph1 = [pmid_pool.tile([128, GT], f32, tag="pmid",
                                      name=f"ph1_{rep}_{g}_{ka}")
                       for ka in range(KA)]
                for kd in range(KD):
                    for ka in range(KA):
                        nc.tensor.matmul(
                            ph1[ka][:],
                            dwt[:, kd, ka * 128:(ka + 1) * 128],
                            xTg[:, kd, :],
                            start=(kd == 0), stop=(kd == KD - 1),
                        )

                gt_ = gh_pool.tile([128, KA, GT], bf16, tag="gt",
                                   name=f"gt_{rep}_{g}")
                for ka in range(KA):
                    nc.scalar.activation(
                        gt_[:, ka, :], ph1[ka][:], AF.Gelu,
                        bias=db[:, ka:ka + 1],
                    )

                ph2 = [pmid_pool.tile([128, GT], f32, tag="pmid",
                                      name=f"ph2_{rep}_{g}_{ka}")
                       for ka in range(KA)]
                for ki in range(KA):
                    for ao in range(KA):
                        nc.tensor.matmul(
                            ph2[ao][:],
                            wet[:, s, ki, ao * 128:(ao + 1) * 128],
                            gt_[:, ki, :],
                            start=(ki == 0), stop=(ki == KA - 1),
                        )

                h3 = gh_pool.tile([128, KA, GT], bf16, tag="h3",
                                  name=f"h3_{rep}_{g}")
                for ao in range(KA):
                    nc.scalar.activation(
                        h3[:, ao, :], ph2[ao][:], AF.Identity,
                        bias=beme[:, s * KA + ao:s * KA + ao + 1],
                    )
                state[(rep, g)] = (xTg, h3)

            def back(rep, g):
                xTg, h3 = state.pop((rep, g))
                outt = out_pool.tile([128, TS, D], bf16, tag="outt",
                                     name=f"outt_{rep}_{g}")
                for ts in range(TS):
                    pxs = []
                    for dh in range(DH):
                        px = px_pool.tile([128, 512], f32, tag="px",
                                          name=f"px_{rep}_{g}_{ts}_{dh}")
                        pxs.append(px)
                        for k in range(KA):
                            nc.tensor.matmul(
                                px[:],
                                h3[:, k, ts * 128:(ts + 1) * 128],
                                upt[:, k, dh * 512:(dh + 1) * 512],
                                start=(k == 0), stop=False,
                            )
                        for j in range(4):
                            kd = dh * 4 + j
                            nc.tensor.matmul(
                                px[:, j * 128:(j + 1) * 128],
                                xTg[:, kd, ts * 128:(ts + 1) * 128],
                                ident[:],
                                start=False, stop=(j == 3),
                            )

                    st = st_pool.tile([128, 12], f32, tag="st",
                                      name=f"st_{rep}_{g}_{ts}")
                    nc.vector.bn_stats(st[:, 0:6], pxs[0][:])
                    nc.vector.bn_stats(st[:, 6:12], pxs[1][:])
                    mv = st_pool.tile([128, 2], f32, tag="mv",
                                      name=f"mv_{rep}_{g}_{ts}")
                    nc.vector.bn_aggr(mv[:], st[:])
                    varv = mv[:, 1:2]
                    vhn = st_pool.tile([128, 1], f32, tag="vhn",
                                       name=f"vhn_{rep}_{g}_{ts}")
                    nc.vector.tensor_scalar(vhn[:], varv, -0.5, -0.5 * LN_EPS,
                                            ALU.mult, ALU.add)
                    yj = st_pool.tile([128, 1], i32, tag="yj",
                                      name=f"yj_{rep}_{g}_{ts}")
                    nc.vector.tensor_scalar(yj[:], varv.bitcast(i32), 1, None,
                                            ALU.logical_shift_right)
                    rs = st_pool.tile([128, 1], f32, tag="rs",
                                      name=f"rs_{rep}_{g}_{ts}")
                    nc.vector.tensor_scalar(rs[:].bitcast(i32), yj[:], -1, FISR_C,
                                            ALU.mult, ALU.add)
                    q = st_pool.tile([128, 1], f32, tag="q",
                                     name=f"q_{rep}_{g}_{ts}")
                    tt = st_pool.tile([128, 1], f32, tag="tt",
                                      name=f"tt_{rep}_{g}_{ts}")
                    for _ in range(2):
                        nc.vector.tensor_mul(q[:], rs[:], rs[:])
                        nc.vector.tensor_mul(tt[:], q[:], vhn[:])
                        nc.vector.tensor_scalar(tt[:], tt[:], 1.5, None, ALU.add)
                        nc.vector.tensor_mul(rs[:], rs[:], tt[:])
                    nmr = st_pool.tile([128, 1], f32, tag="nmr",
                                       name=f"nmr_{rep}_{g}_{ts}")
                    nc.vector.tensor_mul(nmr[:], mv[:, 0:1], rs[:])
                    nc.vector.tensor_scalar_mul(nmr[:], nmr[:], -1.0)

                    for dh in range(DH):
                        nc.scalar.activation(
                            outt[:, ts, dh * 512:(dh + 1) * 512], pxs[dh][:],
                            AF.Identity, bias=nmr[:], scale=rs[:],
                        )
                    nc.sync.dma_start(
                        Y[g * GT + ts * 128:g * GT + (ts + 1) * 128, :],
                        outt[:, ts, :],
                    )

            total = repeat * NG
            for k in range(total + 1):
                if k < total:
                    front(k // NG, k % NG)
                if k >= 1:
                    back((k - 1) // NG, (k - 1) % NG)

    nc.compile()
    return nc


class _Runner:
    """jit-once PJRT runner for the SPMD kernel (axon path)."""

    def __init__(self, nc, n_cores):
        import jax
        from jax.sharding import Mesh, PartitionSpec
        from jax.experimental.shard_map import shard_map

        install_neuronx_cc_hook()
        self.nc = nc
        self.n_cores = n_cores
        pname = nc.partition_id_tensor.name if nc.partition_id_tensor else None

        in_names, out_names, out_avals, zero_outs = [], [], [], []
        for alloc in nc.m.functions[0].allocations:
            if not isinstance(alloc, mybir.MemoryLocationSet):
                continue
            name = alloc.memorylocations[0].name
            if alloc.kind == "ExternalInput":
                if name != pname:
                    in_names.append(name)
            elif alloc.kind == "ExternalOutput":
                out_names.append(name)
                shape = tuple(alloc.tensor_shape)
                dtype = mybir.dt.np(alloc.dtype)
                out_avals.append(jax.core.ShapedArray(shape, dtype))
                zero_outs.append(np.zeros(shape, dtype))
        self.in_names = in_names
        self.out_names = out_names
        self.zero_outs = zero_outs
        n_params = len(in_names)
        n_outs = len(out_avals)
        all_in = list(in_names) + list(out_names)
        if pname is not None:
            all_in.append(pname)

        def _body(*args):
            operands = list(args)
            if pname is not None:
                operands.append(partition_id_tensor())
            outs = _bass_exec_p.bind(
                *operands,
                out_avals=tuple(out_avals),
                in_names=tuple(all_in),
                out_names=tuple(out_names),
                lowering_input_output_aliases=(),
                sim_require_finite=True,
                sim_require_nnan=True,
                nc=nc,
            )
            return tuple(outs)

        devices = jax.devices()[:n_cores]
        mesh = Mesh(np.asarray(devices), ("core",))
        in_specs = (PartitionSpec("core"),) * (n_params + n_outs)
        out_specs = (PartitionSpec("core"),) * n_outs
        self._fn = jax.jit(
            shard_map(_body, mesh=mesh, in_specs=in_specs,
                      out_specs=out_specs, check_rep=False),
            keep_unused=True,
        )

    def run_concat(self, concat_map):
        """concat_map: name -> np array with per-core blocks stacked on axis 0."""
        args = [concat_map[k] for k in self.in_names]
        zeros = [np.concatenate([z] * self.n_cores, axis=0) for z in self.zero_outs]
        outs = self._fn(*args, *zeros)
        return {name: np.asarray(o) for name, o in zip(self.out_names, outs)}


_RUNNER_CACHE = {}


def _get_runner(repeat=1):
    key = repeat
    if key not in _RUNNER_CACHE:
        _RUNNER_CACHE[key] = _Runner(_build_nc(repeat=repeat), N_CORES)
    return _RUNNER_CACHE[key]


def _prep_concat(hidden_states, domain_id, down_W, down_b, up_W, up_b,
                 expert_W, expert_b, domain_emb):
    import ml_dtypes
    bf = ml_dtypes.bfloat16

    hs = np.asarray(hidden_states, dtype=np.float32)
    dom = np.asarray(domain_id)
    valid = (dom >= 0) & (dom < E)
    idx = np.clip(dom, 0, E - 1).astype(np.int64)

    down_W = np.asarray(down_W, dtype=np.float32)
    down_b = np.asarray(down_b, dtype=np.float32)
    up_W = np.asarray(up_W, dtype=np.float32)
    up_b = np.asarray(up_b, dtype=np.float32)
    expert_W = np.asarray(expert_W, dtype=np.float32)
    expert_b = np.asarray(expert_b, dtype=np.float32)
    domain_emb = np.asarray(domain_emb, dtype=np.float32)

    # per-sample expert gather on host (invalid -> identity + emb only)
    wet = np.empty((B, A, A), bf)
    beme = np.empty((B, A), np.float32)
    eye = np.eye(A, dtype=np.float32)
    for b in range(B):
        if valid[b]:
            wet[b] = expert_W[idx[b]].T.astype(bf)
            beme[b] = expert_b[idx[b]] + domain_emb[idx[b]]
        else:
            wet[b] = eye.astype(bf)
            beme[b] = domain_emb[idx[b]]

    # fold up_b into the residual stream; correct the down-proj bias so the
    # adapter input is unchanged: (x+ub) @ dwT + (db - ub @ dwT) == x@dwT + db
    dbp = down_b - down_W @ up_b

    # transposed (x + up_b) per core, group-major so each per-group DMA reads
    # 8KB-contiguous partition rows: [NG, 128p, KD, GT], stacked on axis 0
    hs2 = (hs + up_b).reshape(N_CORES, T, D)
    xt = np.empty((N_CORES * NG, 128, KD, GT), bf)
    for c in range(N_CORES):
        xT = hs2[c].T.astype(bf)  # [D, T]
        xt[c * NG:(c + 1) * NG] = (
            xT.reshape(KD, 128, NG, GT).transpose(2, 1, 0, 3))

    dwT = np.ascontiguousarray(down_W.T).astype(bf)
    db = np.ascontiguousarray(dbp.reshape(KA, 128).T)
    upT = np.ascontiguousarray(up_W.T).astype(bf)
    ident = np.eye(128, dtype=bf)
    beme_t = np.stack([
        np.ascontiguousarray(
            beme[c * NS:(c + 1) * NS].reshape(NS * KA, 128).T)
        for c in range(N_CORES)
    ])  # [8, 128, NS*KA]

    concat = {
        "xt": xt,
        "dwt": np.concatenate([dwT] * N_CORES, axis=0),
        "db": np.concatenate([db] * N_CORES, axis=0),
        "wet": wet.reshape(N_CORES * NS, A, A),
        "beme": beme_t.reshape(N_CORES * 128, NS * KA),
        "upt": np.concatenate([upT] * N_CORES, axis=0),
        "ident": np.concatenate([ident] * N_CORES, axis=0),
    }
    return concat


def kernel(hidden_states, domain_id, down_W, down_b, up_W, up_b,
           expert_W, expert_b, domain_emb, ln_gamma, ln_beta):
    concat = _prep_concat(hidden_states, domain_id, down_W, down_b,
                          up_W, up_b, expert_W, expert_b, domain_emb)
    runner = _get_runner()
    outs = runner.run_concat(concat)
    y = outs["y"].astype(np.float32).reshape(B, S, D)

    g = np.asarray(ln_gamma, dtype=np.float32)
    bta = np.asarray(ln_beta, dtype=np.float32)
    if not (np.all(g == 1.0) and np.all(bta == 0.0)):
        y = y * g + bta
    return y


# revision 17
# speedup vs baseline: 1.0949x; 1.0949x over previous
"""Trainium2 Bass kernel for nn_DomainAdapter (moe_routing).

Reference computation (per sample b with expert e = domain_id[b]):
    h  = gelu(x @ down_W.T + down_b)                  # [S, A]
    h  = where(valid, h @ W_e.T + b_e, h) + emb[e]    # per-sample expert
    y  = LayerNorm(h @ up_W.T + up_b + x) * gamma + beta

Strategy (8 NeuronCores, data-parallel over batch; 4 samples/core):
  - host: gather per-sample expert weights (invalid ids -> identity W, so
    the on-device math is uniform), fold up_b into the residual stream
    (x'' = x + up_b with down_b' = down_b - down_W @ up_b so the down
    projection is unchanged), transpose x'' to xT [D, T] per core, cast
    everything on the matmul path to bf16
  - device: all matmuls in bf16 (1 cycle/row on the PE at any tile size;
    fp32r pays 4x on free-dim<256 outputs, which the residual-transpose
    identity matmuls would otherwise hit)
      down:   h1.T[a,t] = dwT.T @ xT       (xT tiles straight from DRAM)
      gelu:   ACT Gelu LUT (erf-exact) + down_b' as per-partition bias
      expert: per-sample W.T as stationary tiles (compile-time indexed)
      up:     accumulated in PSUM together with the residual (identity
              matmuls from xT chunks; up_b already folded into xT)
      LN:     bn_stats/bn_aggr on DVE, rsqrt via fast-inverse-sqrt + 2
              Newton steps on DVE, normalize on ACT (scale/bias form),
              bf16 output
  - ln_gamma/ln_beta applied on host only if non-trivial (they are 1/0)
"""
import numpy as np

from concourse import bacc, tile, mybir
from concourse.bass2jax import (
    _bass_exec_p,
    install_neuronx_cc_hook,
    partition_id_tensor,
)

f32 = mybir.dt.float32
bf16 = mybir.dt.bfloat16
i32 = mybir.dt.int32
AF = mybir.ActivationFunctionType
ALU = mybir.AluOpType

B, S, D, A, E = 32, 1024, 1024, 256, 16
N_CORES = 8
NS = B // N_CORES          # samples per core = 4
T = NS * S                 # tokens per core = 4096
GT = 512                   # tokens per group
NG = T // GT               # 8 groups
TS = 4                     # 128-token sub-tiles per group
KD = D // 128              # 8
KA = A // 128              # 2
DH = D // 512              # 2
LN_EPS = 1e-5
FISR_C = 0x5F3759DF


def _build_nc(repeat=1):
    nc = bacc.Bacc("TRN2", target_bir_lowering=False, debug=False)

    XT = nc.dram_tensor("xt", [NG, 128, KD, GT], bf16, kind="ExternalInput").ap()
    DWT = nc.dram_tensor("dwt", [D, A], bf16, kind="ExternalInput").ap()
    DB = nc.dram_tensor("db", [128, KA], f32, kind="ExternalInput").ap()
    WET = nc.dram_tensor("wet", [NS, A, A], bf16, kind="ExternalInput").ap()
    BEME = nc.dram_tensor("beme", [128, NS * KA], f32, kind="ExternalInput").ap()
    UPT = nc.dram_tensor("upt", [A, D], bf16, kind="ExternalInput").ap()
    IDENT = nc.dram_tensor("ident", [128, 128], bf16, kind="ExternalInput").ap()
    Y = nc.dram_tensor("y", [T, D], bf16, kind="ExternalOutput").ap()

    with tile.TileContext(nc) as tc:
        with (
            tc.tile_pool(name="consts", bufs=1) as cpool,
            tc.tile_pool(name="xTp", bufs=6) as xT_pool,
            tc.tile_pool(name="gh", bufs=3) as gh_pool,
            tc.tile_pool(name="outp", bufs=3) as out_pool,
            tc.tile_pool(name="stats", bufs=10) as st_pool,
            tc.tile_pool(name="pmid", bufs=2, space="PSUM") as pmid_pool,
            tc.tile_pool(name="px", bufs=6, space="PSUM") as px_pool,
        ):
            dwt = cpool.tile([128, KD, A], bf16, tag="dwt")
            dwt_src = DWT.rearrange("(k p) a -> p k a", p=128)
            nc.sync.dma_start(dwt[:, 0:2], dwt_src[:, 0:2])
            nc.sync.dma_start(dwt[:, 2:KD], dwt_src[:, 2:KD])
            db = cpool.tile([128, KA], f32, tag="db")
            nc.scalar.dma_start(db[:], DB)
            ident = cpool.tile([128, 128], bf16, tag="ident")
            nc.scalar.dma_start(ident[:], IDENT)
            wet = cpool.tile([128, NS, KA, A], bf16, tag="wet")
            nc.sync.dma_start(wet[:], WET.rearrange("s (k p) a -> p s k a", p=128))
            upt = cpool.tile([128, KA, D], bf16, tag="upt")
            nc.sync.dma_start(upt[:], UPT.rearrange("(k p) d -> p k d", p=128))
            beme = cpool.tile([128, NS * KA], f32, tag="beme")
            nc.scalar.dma_start(beme[:], BEME)

            state = {}

            def front(rep, g):
                s = g // 2
                xTg = xT_pool.tile([128, KD, GT], bf16, tag="xTg",
                                   name=f"xTg_{rep}_{g}")
                if rep == 0 and g == 0:
                    # split the very first load per contraction chunk so the
                    # first down matmuls start ~3.5us earlier
                    for kd in range(KD):
                        nc.gpsimd.dma_start(xTg[:, kd:kd + 1], XT[g, :, kd:kd + 1])
                else:
                    nc.gpsimd.dma_start(xTg[:], XT[g])

                ph1 = [pmid_pool.tile([128, GT], f32, tag="pmid",
                                      name=f"ph1_{rep}_{g}_{ka}")
                       for ka in range(KA)]
                for kd in range(KD):
                    for ka in range(KA):
                        nc.tensor.matmul(
                            ph1[ka][:],
                            dwt[:, kd, ka * 128:(ka + 1) * 128],
                            xTg[:, kd, :],
                            start=(kd == 0), stop=(kd == KD - 1),
                        )

                gt_ = gh_pool.tile([128, KA, GT], bf16, tag="gt",
                                   name=f"gt_{rep}_{g}")
                for ka in range(KA):
                    nc.scalar.activation(
                        gt_[:, ka, :], ph1[ka][:], AF.Gelu,
                        bias=db[:, ka:ka + 1],
                    )

                ph2 = [pmid_pool.tile([128, GT], f32, tag="pmid",
                                      name=f"ph2_{rep}_{g}_{ka}")
                       for ka in range(KA)]
                for ki in range(KA):
                    for ao in range(KA):
                        nc.tensor.matmul(
                            ph2[ao][:],
                            wet[:, s, ki, ao * 128:(ao + 1) * 128],
                            gt_[:, ki, :],
                            start=(ki == 0), stop=(ki == KA - 1),
                        )

                h3 = gh_pool.tile([128, KA, GT], bf16, tag="h3",
                                  name=f"h3_{rep}_{g}")
                for ao in range(KA):
                    nc.scalar.activation(
                        h3[:, ao, :], ph2[ao][:], AF.Identity,
                        bias=beme[:, s * KA + ao:s * KA + ao + 1],
                    )
                state[(rep, g)] = (xTg, h3)

            def back(rep, g):
                xTg, h3 = state.pop((rep, g))
                outt = out_pool.tile([128, TS, D], bf16, tag="outt",
                                     name=f"outt_{rep}_{g}")
                for ts in range(TS):
                    pxs = []
                    for dh in range(DH):
                        px = px_pool.tile([128, 512], f32, tag="px",
                                          name=f"px_{rep}_{g}_{ts}_{dh}")
                        pxs.append(px)
                        for k in range(KA):
                            nc.tensor.matmul(
                                px[:],
                                h3[:, k, ts * 128:(ts + 1) * 128],
                                upt[:, k, dh * 512:(dh + 1) * 512],
                                start=(k == 0), stop=False,
                            )
                        for j in range(4):
                            kd = dh * 4 + j
                            nc.tensor.matmul(
                                px[:, j * 128:(j + 1) * 128],
                                xTg[:, kd, ts * 128:(ts + 1) * 128],
                                ident[:],
                                start=False, stop=(j == 3),
                            )

                    st = st_pool.tile([128, 12], f32, tag="st",
                                      name=f"st_{rep}_{g}_{ts}")
                    nc.vector.bn_stats(st[:, 0:6], pxs[0][:])
                    nc.vector.bn_stats(st[:, 6:12], pxs[1][:])
                    mv = st_pool.tile([128, 2], f32, tag="mv",
                                      name=f"mv_{rep}_{g}_{ts}")
                    nc.vector.bn_aggr(mv[:], st[:])
                    varv = mv[:, 1:2]
                    vhn = st_pool.tile([128, 1], f32, tag="vhn",
                                       name=f"vhn_{rep}_{g}_{ts}")
                    nc.vector.tensor_scalar(vhn[:], varv, -0.5, -0.5 * LN_EPS,
                                            ALU.mult, ALU.add)
                    yj = st_pool.tile([128, 1], i32, tag="yj",
                                      name=f"yj_{rep}_{g}_{ts}")
                    nc.vector.tensor_scalar(yj[:], varv.bitcast(i32), 1, None,
                                            ALU.logical_shift_right)
                    rs = st_pool.tile([128, 1], f32, tag="rs",
                                      name=f"rs_{rep}_{g}_{ts}")
                    nc.vector.tensor_scalar(rs[:].bitcast(i32), yj[:], -1, FISR_C,
                                            ALU.mult, ALU.add)
                    q = st_pool.tile([128, 1], f32, tag="q",
                                     name=f"q_{rep}_{g}_{ts}")
                    tt = st_pool.tile([128, 1], f32, tag="tt",
                                      name=f"tt_{rep}_{g}_{ts}")
                    for _ in range(2):
                        nc.vector.tensor_mul(q[:], rs[:], rs[:])
                        nc.vector.tensor_mul(tt[:], q[:], vhn[:])
                        nc.vector.tensor_scalar(tt[:], tt[:], 1.5, None, ALU.add)
                        nc.vector.tensor_mul(rs[:], rs[:], tt[:])
                    nmr = st_pool.tile([128, 1], f32, tag="nmr",
                                       name=f"nmr_{rep}_{g}_{ts}")
                    nc.vector.tensor_mul(nmr[:], mv[:, 0:1], rs[:])
                    nc.vector.tensor_scalar_mul(nmr[:], nmr[:], -1.0)

                    for dh in range(DH):
                        nc.scalar.activation(
                            outt[:, ts, dh * 512:(dh + 1) * 512], pxs[dh][:],
                            AF.Identity, bias=nmr[:], scale=rs[:],
                        )
                    nc.sync.dma_start(
                        Y[g * GT + ts * 128:g * GT + (ts + 1) * 128, :],
                        outt[:, ts, :],
                    )

            total = repeat * NG
            for k in range(total + 1):
                if k < total:
                    front(k // NG, k % NG)
                if k >= 1:
                    back((k - 1) // NG, (k - 1) % NG)

    nc.compile()
    return nc


class _Runner:
    """jit-once PJRT runner for the SPMD kernel (axon path)."""

    def __init__(self, nc, n_cores):
        import jax
        from jax.sharding import Mesh, PartitionSpec
        from jax.experimental.shard_map import shard_map

        install_neuronx_cc_hook()
        self.nc = nc
        self.n_cores = n_cores
        pname = nc.partition_id_tensor.name if nc.partition_id_tensor else None

        in_names, out_names, out_avals, zero_outs = [], [], [], []
        for alloc in nc.m.functions[0].allocations:
            if not isinstance(alloc, mybir.MemoryLocationSet):
                continue
            name = alloc.memorylocations[0].name
            if alloc.kind == "ExternalInput":
                if name != pname:
                    in_names.append(name)
            elif alloc.kind == "ExternalOutput":
                out_names.append(name)
                shape = tuple(alloc.tensor_shape)
                dtype = mybir.dt.np(alloc.dtype)
                out_avals.append(jax.core.ShapedArray(shape, dtype))
                zero_outs.append(np.zeros(shape, dtype))
        self.in_names = in_names
        self.out_names = out_names
        self.zero_outs = zero_outs
        n_params = len(in_names)
        n_outs = len(out_avals)
        all_in = list(in_names) + list(out_names)
        if pname is not None:
            all_in.append(pname)

        def _body(*args):
            operands = list(args)
            if pname is not None:
                operands.append(partition_id_tensor())
            outs = _bass_exec_p.bind(
                *operands,
                out_avals=tuple(out_avals),
                in_names=tuple(all_in),
                out_names=tuple(out_names),
                lowering_input_output_aliases=(),
                sim_require_finite=True,
                sim_require_nnan=True,
                nc=nc,
            )
            return tuple(outs)

        devices = jax.devices()[:n_cores]
        mesh = Mesh(np.asarray(devices), ("core",))
        in_specs = (PartitionSpec("core"),) * (n_params + n_outs)
        out_specs = (PartitionSpec("core"),) * n_outs
        self._fn = jax.jit(
            shard_map(_body, mesh=mesh, in_specs=in_specs,
                      out_specs=out_specs, check_rep=False),
            keep_unused=True,
        )

    def run_concat(self, concat_map):
        """concat_map: name -> np array with per-core blocks stacked on axis 0."""
        args = [concat_map[k] for k in self.in_names]
        zeros = [np.concatenate([z] * self.n_cores, axis=0) for z in self.zero_outs]
        outs = self._fn(*args, *zeros)
        return {name: np.asarray(o) for name, o in zip(self.out_names, outs)}


_RUNNER_CACHE = {}


def _get_runner(repeat=1):
    key = repeat
    if key not in _RUNNER_CACHE:
        _RUNNER_CACHE[key] = _Runner(_build_nc(repeat=repeat), N_CORES)
    return _RUNNER_CACHE[key]


def _prep_concat(hidden_states, domain_id, down_W, down_b, up_W, up_b,
                 expert_W, expert_b, domain_emb):
    import ml_dtypes
    bf = ml_dtypes.bfloat16

    hs = np.asarray(hidden_states, dtype=np.float32)
    dom = np.asarray(domain_id)
    valid = (dom >= 0) & (dom < E)
    idx = np.clip(dom, 0, E - 1).astype(np.int64)

    down_W = np.asarray(down_W, dtype=np.float32)
    down_b = np.asarray(down_b, dtype=np.float32)
    up_W = np.asarray(up_W, dtype=np.float32)
    up_b = np.asarray(up_b, dtype=np.float32)
    expert_W = np.asarray(expert_W, dtype=np.float32)
    expert_b = np.asarray(expert_b, dtype=np.float32)
    domain_emb = np.asarray(domain_emb, dtype=np.float32)

    # per-sample expert gather on host (invalid -> identity + emb only)
    wet = np.empty((B, A, A), bf)
    beme = np.empty((B, A), np.float32)
    eye = np.eye(A, dtype=np.float32)
    for b in range(B):
        if valid[b]:
            wet[b] = expert_W[idx[b]].T.astype(bf)
            beme[b] = expert_b[idx[b]] + domain_emb[idx[b]]
        else:
            wet[b] = eye.astype(bf)
            beme[b] = domain_emb[idx[b]]

    # fold up_b into the residual stream; correct the down-proj bias so the
    # adapter input is unchanged: (x+ub) @ dwT + (db - ub @ dwT) == x@dwT + db
    dbp = down_b - down_W @ up_b

    # transposed (x + up_b) per core, group-major so each per-group DMA reads
    # 8KB-contiguous partition rows: [NG, 128p, KD, GT], stacked on axis 0
    hs2 = (hs + up_b).reshape(N_CORES, T, D)
    xt = np.empty((N_CORES * NG, 128, KD, GT), bf)
    for c in range(N_CORES):
        xT = hs2[c].T.astype(bf)  # [D, T]
        xt[c * NG:(c + 1) * NG] = (
            xT.reshape(KD, 128, NG, GT).transpose(2, 1, 0, 3))

    dwT = np.ascontiguousarray(down_W.T).astype(bf)
    db = np.ascontiguousarray(dbp.reshape(KA, 128).T)
    upT = np.ascontiguousarray(up_W.T).astype(bf)
    ident = np.eye(128, dtype=bf)
    beme_t = np.stack([
        np.ascontiguousarray(
            beme[c * NS:(c + 1) * NS].reshape(NS * KA, 128).T)
        for c in range(N_CORES)
    ])  # [8, 128, NS*KA]

    concat = {
        "xt": xt,
        "dwt": np.concatenate([dwT] * N_CORES, axis=0),
        "db": np.concatenate([db] * N_CORES, axis=0),
        "wet": wet.reshape(N_CORES * NS, A, A),
        "beme": beme_t.reshape(N_CORES * 128, NS * KA),
        "upt": np.concatenate([upT] * N_CORES, axis=0),
        "ident": np.concatenate([ident] * N_CORES, axis=0),
    }
    return concat


def kernel(hidden_states, domain_id, down_W, down_b, up_W, up_b,
           expert_W, expert_b, domain_emb, ln_gamma, ln_beta):
    concat = _prep_concat(hidden_states, domain_id, down_W, down_b,
                          up_W, up_b, expert_W, expert_b, domain_emb)
    runner = _get_runner()
    outs = runner.run_concat(concat)
    y = outs["y"].astype(np.float32).reshape(B, S, D)

    g = np.asarray(ln_gamma, dtype=np.float32)
    bta = np.asarray(ln_beta, dtype=np.float32)
    if not (np.all(g == 1.0) and np.all(bta == 0.0)):
        y = y * g + bta
    return y


# revision 22
# speedup vs baseline: 1.3589x; 1.2411x over previous
"""Trainium2 Bass kernel for nn_DomainAdapter (moe_routing).

Reference computation (per sample b with expert e = domain_id[b]):
    h  = gelu(x @ down_W.T + down_b)                  # [S, A]
    h  = where(valid, h @ W_e.T + b_e, h) + emb[e]    # per-sample expert
    y  = LayerNorm(h @ up_W.T + up_b + x) * gamma + beta

Strategy (8 NeuronCores, data-parallel over batch; 4 samples/core):
  - host: gather per-sample expert weights (invalid ids -> identity W, so
    the on-device math is uniform), fold up_b into the residual stream
    (x'' = x + up_b with down_b' = down_b - down_W @ up_b so the down
    projection is unchanged), transpose x'' to xT [D, T] per core, cast
    everything on the matmul path to bf16
  - device: all matmuls in bf16 (1 cycle/row on the PE at any tile size;
    fp32r pays 4x on free-dim<256 outputs, which the residual-transpose
    identity matmuls would otherwise hit)
      down:   h1.T[a,t] = dwT.T @ xT       (xT tiles straight from DRAM)
      gelu:   ACT Gelu LUT (erf-exact) + down_b' as per-partition bias
      expert: per-sample W.T as stationary tiles (compile-time indexed)
      up:     accumulated in PSUM together with the residual (identity
              matmuls from xT chunks; up_b already folded into xT)
      LN:     bn_stats/bn_aggr on DVE, rsqrt via fast-inverse-sqrt + 2
              Newton steps on DVE, normalize on ACT (scale/bias form),
              bf16 output
  - ln_gamma/ln_beta applied on host only if non-trivial (they are 1/0)
"""
import numpy as np

from concourse import bacc, tile, mybir
from concourse.bass2jax import (
    _bass_exec_p,
    install_neuronx_cc_hook,
    partition_id_tensor,
)

f32 = mybir.dt.float32
bf16 = mybir.dt.bfloat16
i32 = mybir.dt.int32
AF = mybir.ActivationFunctionType
ALU = mybir.AluOpType

B, S, D, A, E = 32, 1024, 1024, 256, 16
N_CORES = 8
NS = B // N_CORES          # samples per core = 4
T = NS * S                 # tokens per core = 4096
GT = 512                   # tokens per group
NG = T // GT               # 8 groups
TS = 4                     # 128-token sub-tiles per group
KD = D // 128              # 8
KA = A // 128              # 2
DH = D // 512              # 2
LN_EPS = 1e-5
FISR_C = 0x5F3759DF


def _build_nc(repeat=1):
    nc = bacc.Bacc("TRN2", target_bir_lowering=False, debug=False)

    XT = nc.dram_tensor("xt", [NG, 128, KD, GT], bf16, kind="ExternalInput").ap()
    DWT = nc.dram_tensor("dwt", [D, A], bf16, kind="ExternalInput").ap()
    DB = nc.dram_tensor("db", [128, KA], f32, kind="ExternalInput").ap()
    WET = nc.dram_tensor("wet", [NS, A, A], bf16, kind="ExternalInput").ap()
    BEME = nc.dram_tensor("beme", [128, NS * KA], f32, kind="ExternalInput").ap()
    UPT = nc.dram_tensor("upt", [A, D], bf16, kind="ExternalInput").ap()
    IDENT = nc.dram_tensor("ident", [128, 128], bf16, kind="ExternalInput").ap()
    Y = nc.dram_tensor("y", [T, D], bf16, kind="ExternalOutput").ap()

    with tile.TileContext(nc) as tc:
        with (
            tc.tile_pool(name="consts", bufs=1) as cpool,
            tc.tile_pool(name="xTp", bufs=6) as xT_pool,
            tc.tile_pool(name="gh", bufs=3) as gh_pool,
            tc.tile_pool(name="outp", bufs=3) as out_pool,
            tc.tile_pool(name="stats", bufs=10) as st_pool,
            tc.tile_pool(name="pmid", bufs=2, space="PSUM") as pmid_pool,
            tc.tile_pool(name="px", bufs=6, space="PSUM") as px_pool,
        ):
            dwt = cpool.tile([128, KD, A], bf16, tag="dwt")
            dwt_src = DWT.rearrange("(k p) a -> p k a", p=128)
            nc.sync.dma_start(dwt[:, 0:2], dwt_src[:, 0:2])
            nc.sync.dma_start(dwt[:, 2:KD], dwt_src[:, 2:KD])
            db = cpool.tile([128, KA], f32, tag="db")
            nc.scalar.dma_start(db[:], DB)
            ident = cpool.tile([128, 128], bf16, tag="ident")
            nc.scalar.dma_start(ident[:], IDENT)
            wet = cpool.tile([128, NS, KA, A], bf16, tag="wet")
            nc.sync.dma_start(wet[:], WET.rearrange("s (k p) a -> p s k a", p=128))
            upt = cpool.tile([128, KA, D], bf16, tag="upt")
            nc.sync.dma_start(upt[:], UPT.rearrange("(k p) d -> p k d", p=128))
            beme = cpool.tile([128, NS * KA], f32, tag="beme")
            nc.scalar.dma_start(beme[:], BEME)
            epsb = cpool.tile([128, 1], f32, tag="epsb")
            nc.vector.memset(epsb[:], LN_EPS)

            state = {}

            def front(rep, g):
                s = g // 2
                xTg = xT_pool.tile([128, KD, GT], bf16, tag="xTg",
                                   name=f"xTg_{rep}_{g}")
                if rep == 0 and g == 0:
                    # split the very first load per contraction chunk so the
                    # first down matmuls start ~3.5us earlier
                    for kd in range(KD):
                        nc.gpsimd.dma_start(xTg[:, kd:kd + 1], XT[g, :, kd:kd + 1])
                else:
                    nc.gpsimd.dma_start(xTg[:], XT[g])

                ph1 = [pmid_pool.tile([128, GT], f32, tag="pmid",
                                      name=f"ph1_{rep}_{g}_{ka}")
                       for ka in range(KA)]
                for kd in range(KD):
                    for ka in range(KA):
                        nc.tensor.matmul(
                            ph1[ka][:],
                            dwt[:, kd, ka * 128:(ka + 1) * 128],
                            xTg[:, kd, :],
                            start=(kd == 0), stop=(kd == KD - 1),
                        )

                gt_ = gh_pool.tile([128, KA, GT], bf16, tag="gt",
                                   name=f"gt_{rep}_{g}")
                for ka in range(KA):
                    nc.scalar.activation(
                        gt_[:, ka, :], ph1[ka][:], AF.Gelu,
                        bias=db[:, ka:ka + 1],
                    )

                ph2 = [pmid_pool.tile([128, GT], f32, tag="pmid",
                                      name=f"ph2_{rep}_{g}_{ka}")
                       for ka in range(KA)]
                for ki in range(KA):
                    for ao in range(KA):
                        nc.tensor.matmul(
                            ph2[ao][:],
                            wet[:, s, ki, ao * 128:(ao + 1) * 128],
                            gt_[:, ki, :],
                            start=(ki == 0), stop=(ki == KA - 1),
                        )

                h3 = gh_pool.tile([128, KA, GT], bf16, tag="h3",
                                  name=f"h3_{rep}_{g}")
                for ao in range(KA):
                    nc.scalar.activation(
                        h3[:, ao, :], ph2[ao][:], AF.Identity,
                        bias=beme[:, s * KA + ao:s * KA + ao + 1],
                    )
                state[(rep, g)] = (xTg, h3)

            def back(rep, g):
                xTg, h3 = state.pop((rep, g))
                outt = out_pool.tile([128, TS, D], bf16, tag="outt",
                                     name=f"outt_{rep}_{g}")
                for ts in range(TS):
                    pxs = [px_pool.tile([128, 512], f32, tag="px",
                                        name=f"px_{rep}_{g}_{ts}_{dh}")
                           for dh in range(DH)]
                    # k outer / dh inner: each h3 stationary chunk is loaded
                    # into the PE once and pumped for both dh halves
                    for k in range(KA):
                        for dh in range(DH):
                            nc.tensor.matmul(
                                pxs[dh][:],
                                h3[:, k, ts * 128:(ts + 1) * 128],
                                upt[:, k, dh * 512:(dh + 1) * 512],
                                start=(k == 0), stop=False,
                            )
                    for dh in range(DH):
                        for j in range(4):
                            kd = dh * 4 + j
                            nc.tensor.matmul(
                                pxs[dh][:, j * 128:(j + 1) * 128],
                                xTg[:, kd, ts * 128:(ts + 1) * 128],
                                ident[:],
                                start=False, stop=(j == 3),
                            )

                    st = st_pool.tile([128, 12], f32, tag="st",
                                      name=f"st_{rep}_{g}_{ts}")
                    nc.vector.bn_stats(st[:, 0:6], pxs[0][:])
                    nc.vector.bn_stats(st[:, 6:12], pxs[1][:])
                    mv = st_pool.tile([128, 2], f32, tag="mv",
                                      name=f"mv_{rep}_{g}_{ts}")
                    nc.vector.bn_aggr(mv[:], st[:])
                    varv = mv[:, 1:2]
                    sq = st_pool.tile([128, 1], f32, tag="sq",
                                      name=f"sq_{rep}_{g}_{ts}")
                    nc.scalar.activation(sq[:], varv, AF.Sqrt,
                                         bias=epsb[:, 0:1])
                    rs = st_pool.tile([128, 1], f32, tag="rs",
                                      name=f"rs_{rep}_{g}_{ts}")
                    nc.vector.reciprocal(rs[:], sq[:])
                    nmr = st_pool.tile([128, 1], f32, tag="nmr",
                                       name=f"nmr_{rep}_{g}_{ts}")
                    nc.vector.scalar_tensor_tensor(
                        nmr[:], mv[:, 0:1], -1.0, rs[:],
                        ALU.mult, ALU.mult)

                    for dh in range(DH):
                        nc.scalar.activation(
                            outt[:, ts, dh * 512:(dh + 1) * 512], pxs[dh][:],
                            AF.Identity, bias=nmr[:], scale=rs[:],
                        )
                    nc.sync.dma_start(
                        Y[g * GT + ts * 128:g * GT + (ts + 1) * 128, :],
                        outt[:, ts, :],
                    )

            total = repeat * NG
            for k in range(total + 1):
                if k < total:
                    front(k // NG, k % NG)
                if k >= 1:
                    back((k - 1) // NG, (k - 1) % NG)

    nc.compile()
    return nc


class _Runner:
    """jit-once PJRT runner for the SPMD kernel (axon path)."""

    def __init__(self, nc, n_cores):
        import jax
        from jax.sharding import Mesh, PartitionSpec
        from jax.experimental.shard_map import shard_map

        install_neuronx_cc_hook()
        self.nc = nc
        self.n_cores = n_cores
        pname = nc.partition_id_tensor.name if nc.partition_id_tensor else None

        in_names, out_names, out_avals, zero_outs = [], [], [], []
        for alloc in nc.m.functions[0].allocations:
            if not isinstance(alloc, mybir.MemoryLocationSet):
                continue
            name = alloc.memorylocations[0].name
            if alloc.kind == "ExternalInput":
                if name != pname:
                    in_names.append(name)
            elif alloc.kind == "ExternalOutput":
                out_names.append(name)
                shape = tuple(alloc.tensor_shape)
                dtype = mybir.dt.np(alloc.dtype)
                out_avals.append(jax.core.ShapedArray(shape, dtype))
                zero_outs.append(np.zeros(shape, dtype))
        self.in_names = in_names
        self.out_names = out_names
        self.zero_outs = zero_outs
        n_params = len(in_names)
        n_outs = len(out_avals)
        all_in = list(in_names) + list(out_names)
        if pname is not None:
            all_in.append(pname)

        def _body(*args):
            operands = list(args)
            if pname is not None:
                operands.append(partition_id_tensor())
            outs = _bass_exec_p.bind(
                *operands,
                out_avals=tuple(out_avals),
                in_names=tuple(all_in),
                out_names=tuple(out_names),
                lowering_input_output_aliases=(),
                sim_require_finite=True,
                sim_require_nnan=True,
                nc=nc,
            )
            return tuple(outs)

        devices = jax.devices()[:n_cores]
        mesh = Mesh(np.asarray(devices), ("core",))
        in_specs = (PartitionSpec("core"),) * (n_params + n_outs)
        out_specs = (PartitionSpec("core"),) * n_outs
        self._fn = jax.jit(
            shard_map(_body, mesh=mesh, in_specs=in_specs,
                      out_specs=out_specs, check_rep=False),
            keep_unused=True,
        )

    def run_concat(self, concat_map):
        """concat_map: name -> np array with per-core blocks stacked on axis 0."""
        args = [concat_map[k] for k in self.in_names]
        zeros = [np.concatenate([z] * self.n_cores, axis=0) for z in self.zero_outs]
        outs = self._fn(*args, *zeros)
        return {name: np.asarray(o) for name, o in zip(self.out_names, outs)}


_RUNNER_CACHE = {}


def _get_runner(repeat=1):
    key = repeat
    if key not in _RUNNER_CACHE:
        _RUNNER_CACHE[key] = _Runner(_build_nc(repeat=repeat), N_CORES)
    return _RUNNER_CACHE[key]


def _prep_concat(hidden_states, domain_id, down_W, down_b, up_W, up_b,
                 expert_W, expert_b, domain_emb):
    import ml_dtypes
    bf = ml_dtypes.bfloat16

    hs = np.asarray(hidden_states, dtype=np.float32)
    dom = np.asarray(domain_id)
    valid = (dom >= 0) & (dom < E)
    idx = np.clip(dom, 0, E - 1).astype(np.int64)

    down_W = np.asarray(down_W, dtype=np.float32)
    down_b = np.asarray(down_b, dtype=np.float32)
    up_W = np.asarray(up_W, dtype=np.float32)
    up_b = np.asarray(up_b, dtype=np.float32)
    expert_W = np.asarray(expert_W, dtype=np.float32)
    expert_b = np.asarray(expert_b, dtype=np.float32)
    domain_emb = np.asarray(domain_emb, dtype=np.float32)

    # per-sample expert gather on host (invalid -> identity + emb only)
    wet = np.empty((B, A, A), bf)
    beme = np.empty((B, A), np.float32)
    eye = np.eye(A, dtype=np.float32)
    for b in range(B):
        if valid[b]:
            wet[b] = expert_W[idx[b]].T.astype(bf)
            beme[b] = expert_b[idx[b]] + domain_emb[idx[b]]
        else:
            wet[b] = eye.astype(bf)
            beme[b] = domain_emb[idx[b]]

    # fold up_b into the residual stream; correct the down-proj bias so the
    # adapter input is unchanged: (x+ub) @ dwT + (db - ub @ dwT) == x@dwT + db
    dbp = down_b - down_W @ up_b

    # transposed (x + up_b) per core, group-major so each per-group DMA reads
    # 8KB-contiguous partition rows: [NG, 128p, KD, GT], stacked on axis 0
    hs2 = (hs + up_b).reshape(N_CORES, T, D)
    xt = np.empty((N_CORES * NG, 128, KD, GT), bf)
    for c in range(N_CORES):
        xT = hs2[c].T.astype(bf)  # [D, T]
        xt[c * NG:(c + 1) * NG] = (
            xT.reshape(KD, 128, NG, GT).transpose(2, 1, 0, 3))

    dwT = np.ascontiguousarray(down_W.T).astype(bf)
    db = np.ascontiguousarray(dbp.reshape(KA, 128).T)
    upT = np.ascontiguousarray(up_W.T).astype(bf)
    ident = np.eye(128, dtype=bf)
    beme_t = np.stack([
        np.ascontiguousarray(
            beme[c * NS:(c + 1) * NS].reshape(NS * KA, 128).T)
        for c in range(N_CORES)
    ])  # [8, 128, NS*KA]

    concat = {
        "xt": xt,
        "dwt": np.concatenate([dwT] * N_CORES, axis=0),
        "db": np.concatenate([db] * N_CORES, axis=0),
        "wet": wet.reshape(N_CORES * NS, A, A),
        "beme": beme_t.reshape(N_CORES * 128, NS * KA),
        "upt": np.concatenate([upT] * N_CORES, axis=0),
        "ident": np.concatenate([ident] * N_CORES, axis=0),
    }
    return concat


def kernel(hidden_states, domain_id, down_W, down_b, up_W, up_b,
           expert_W, expert_b, domain_emb, ln_gamma, ln_beta):
    concat = _prep_concat(hidden_states, domain_id, down_W, down_b,
                          up_W, up_b, expert_W, expert_b, domain_emb)
    runner = _get_runner()
    outs = runner.run_concat(concat)
    y = outs["y"].astype(np.float32).reshape(B, S, D)

    g = np.asarray(ln_gamma, dtype=np.float32)
    bta = np.asarray(ln_beta, dtype=np.float32)
    if not (np.all(g == 1.0) and np.all(bta == 0.0)):
        y = y * g + bta
    return y
